# revision 4
# baseline (speedup 1.0000x reference)
"""Additive (Bahdanau) attention kernel for Trainium2, 8 NeuronCores.

score[b,t,k] = v . tanh(W1 @ [h_t;c_t] + W2 @ x_k); beta = softmax_k(score);
z = beta @ x.  B=2, T=512, D=H=V=256.

Sharding: data-parallel over (batch, query-time): core s handles batch s//4,
query rows 128*(s%4) .. 128*(s%4)+127.  x[b], W1, W2, v replicated per need.
No collectives; host concatenates the 8 output shards.

Per-core algorithm (all layouts chosen so reductions land on the right axes):
  s_xT[v',k]  = (x @ W2).T          via PE (needs xT, made by PE transpose)
  s_hcT[v',t] = ([h;c] @ W1).T      via PE
  loop over groups of 8 t:
    DVE  : sum[v', (vt,t,k)] = s_xT[v',k] + s_hcT[v',t]   (tensor_scalar_add,
           per-partition scalar, fp32 2x mode)
    ACT  : tanh over the whole [128, 8192] group tile -> fp16
    PE   : scoresT[k, t] += tanh-chunk[v',k-chunk].T @ v  (tanh chunk is the
           fp16 *stationary* so FWL streams it at 2 elem/cycle; moving operand
           is the tiny v column; output lands at base_partition 0)
  tail: transpose scoresT -> scores[t,k], softmax over k (exp with bias=-max),
        transpose beta, z = betaT.T @ x, scale rows by 1/rowsum, DMA out.
"""

import os
import sys

for _p in ("/opt/trn_rl_repo",):
    if _p not in sys.path and os.path.isdir(_p):
        sys.path.insert(0, _p)

import numpy as np

import concourse.bass as bass
import concourse.bacc as bacc
import concourse.mybir as mybir
from concourse import masks
from concourse.bass_utils import run_bass_kernel_spmd
from concourse.tile import TileContext

B, T, D, H, V = 2, 512, 256, 256, 256
NCORES = 8
TL = T * B // NCORES  # 128 query rows per core
G = 8                 # t rows per main-loop group
NG = TL // G          # 16 groups
FP32 = mybir.dt.float32
FP16 = mybir.dt.float16


def build_program() -> bass.Bass:
    nc = bacc.Bacc()

    x_d = nc.declare_dram_parameter("x", [T, D], FP32, isOutput=False)
    h_d = nc.declare_dram_parameter("h", [TL, H], FP32, isOutput=False)
    c_d = nc.declare_dram_parameter("c", [TL, H], FP32, isOutput=False)
    w1_d = nc.declare_dram_parameter("W1", [2 * H, V], FP32, isOutput=False)
    w2_d = nc.declare_dram_parameter("W2", [D, V], FP32, isOutput=False)
    v_d = nc.declare_dram_parameter("v", [V], FP32, isOutput=False)
    out_d = nc.declare_dram_parameter("out", [TL, D], FP32, isOutput=True)

    with TileContext(nc) as tc:
        with (
            tc.tile_pool(name="const", bufs=1) as cpool,
            tc.tile_pool(name="sums", bufs=2) as sum_pool,
            tc.tile_pool(name="tanhs", bufs=2) as tanh_pool,
            tc.tile_pool(name="psum", bufs=2, space="PSUM") as pp,
            tc.tile_pool(name="psum_long", bufs=1, space="PSUM") as ppl,
        ):
            # ---- load inputs -------------------------------------------------
            x_nat = cpool.tile([128, T // 128, D], FP32)       # [p, kb, d]
            nc.sync.dma_start(x_nat[:], x_d[:, :].rearrange("(n p) d -> p n d", p=128))
            h_t = cpool.tile([128, H], FP32)
            nc.sync.dma_start(h_t[:], h_d[:, :])
            c_t = cpool.tile([128, H], FP32)
            nc.sync.dma_start(c_t[:], c_d[:, :])
            w1_t = cpool.tile([128, 4, V], FP32)               # [p, n, v']
            nc.sync.dma_start(w1_t[:], w1_d[:, :].rearrange("(n p) v -> p n v", p=128))
            w2_t = cpool.tile([128, 2, V], FP32)
            nc.sync.dma_start(w2_t[:], w2_d[:, :].rearrange("(n p) v -> p n v", p=128))
            v_t = cpool.tile([128, 2], FP32)
            nc.sync.dma_start(v_t[:], v_d[:].rearrange("(t p) -> p t", p=128))

            ident = cpool.tile([128, 128], FP32)
            masks.make_identity(nc, ident[:])

            v16 = cpool.tile([128, 2], FP16)
            nc.vector.tensor_copy(v16[:], v_t[:])

            # ---- transposes: xT (d on partitions), hcT (d2 on partitions) ----
            xT = [cpool.tile([128, T], FP32, name=f"xT{i}") for i in range(2)]
            for i in range(2):            # d block
                for j in range(T // 128):  # k block
                    tr = pp.tile([128, 128], FP32, tag="tr")
                    nc.tensor.transpose(tr[:], x_nat[:, j, i * 128:(i + 1) * 128], ident[:])
                    nc.vector.tensor_copy(xT[i][:, j * 128:(j + 1) * 128], tr[:])

            hcT = [cpool.tile([128, 128], FP32, name=f"hcT{n}") for n in range(4)]
            for n, (src, i) in enumerate([(h_t, 0), (h_t, 1), (c_t, 0), (c_t, 1)]):
                tr = pp.tile([128, 128], FP32, tag="tr")
                nc.tensor.transpose(tr[:], src[:, i * 128:(i + 1) * 128], ident[:])
                nc.vector.tensor_copy(hcT[n][:], tr[:])

            # ---- s_xT[v',k] and s_hcT[v',t] ---------------------------------
            sxT = [cpool.tile([128, T], FP32, name=f"sxT{vt}") for vt in range(2)]
            for vt in range(2):
                ps = pp.tile([128, T], FP32, tag="mm")
                for i in range(2):
                    nc.tensor.matmul(
                        ps[:], w2_t[:, i, vt * 128:(vt + 1) * 128], xT[i][:],
                        start=(i == 0), stop=(i == 1),
                    )
                nc.vector.tensor_copy(sxT[vt][:], ps[:])

            shcT = [cpool.tile([128, TL], FP32, name=f"shcT{vt}") for vt in range(2)]
            for vt in range(2):
                ps = pp.tile([128, TL], FP32, tag="mm")
                for n in range(4):
                    nc.tensor.matmul(
                        ps[:], w1_t[:, n, vt * 128:(vt + 1) * 128], hcT[n][:],
                        start=(n == 0), stop=(n == 3),
                    )
                nc.vector.tensor_copy(shcT[vt][:], ps[:])

            # ---- main loop ---------------------------------------------------
            # scoresT_psum[p, kb*128 + t] = score[t, k = kb*128 + p]
            scT = ppl.tile([128, T], FP32)
            for g in range(NG):
                sums = sum_pool.tile([128, 2 * G * T], FP32, tag="sums")
                for vt in range(2):
                    for tl in range(G):
                        t = g * G + tl
                        col = vt * (G * T) + tl * T
                        nc.vector.tensor_scalar_add(
                            sums[:, col:col + T], sxT[vt][:], shcT[vt][:, t:t + 1]
                        )
                th = tanh_pool.tile([128, 2 * G * T], FP16, tag="th")
                nc.scalar.activation(th[:], sums[:], mybir.ActivationFunctionType.Tanh)
                for tl in range(G):
                    t = g * G + tl
                    for kb in range(T // 128):
                        col = kb * 128 + t
                        for vt in range(2):
                            lo = vt * (G * T) + tl * T + kb * 128
                            nc.tensor.matmul(
                                scT[:, col:col + 1],
                                th[:, lo:lo + 128],
                                v16[:, vt:vt + 1],
                                start=(vt == 0), stop=(vt == 1),
                            )

            # ---- softmax + z -------------------------------------------------
            scT_sb = [cpool.tile([128, 128], FP32, name=f"scT_sb{kb}") for kb in range(4)]
            for kb in range(4):
                nc.vector.tensor_copy(scT_sb[kb][:], scT[:, kb * 128:(kb + 1) * 128])
            scores = ppl.tile([128, T], FP32)  # [t, k]
            for kb in range(4):
                nc.tensor.transpose(scores[:, kb * 128:(kb + 1) * 128], scT_sb[kb][:], ident[:])

            negmax = cpool.tile([128, 1], FP32)
            nc.vector.reduce_max(negmax[:], scores[:], axis=mybir.AxisListType.X, negate=True)
            p_sb = cpool.tile([128, T], FP32)
            nc.scalar.activation(
                p_sb[:], scores[:], mybir.ActivationFunctionType.Exp, bias=negmax[:]
            )
            rowsum = cpool.tile([128, 1], FP32)
            nc.vector.reduce_sum(rowsum[:], p_sb[:], axis=mybir.AxisListType.X)
            recip = cpool.tile([128, 1], FP32)
            nc.vector.reciprocal(recip[:], rowsum[:])

            pT = [cpool.tile([128, 128], FP32, name=f"pT{kb}") for kb in range(4)]
            for kb in range(4):
                tr = pp.tile([128, 128], FP32, tag="tr")
                nc.tensor.transpose(tr[:], p_sb[:, kb * 128:(kb + 1) * 128], ident[:])
                nc.vector.tensor_copy(pT[kb][:], tr[:])

            z_ps = pp.tile([128, D], FP32, tag="mm")
            for kb in range(4):
                nc.tensor.matmul(
                    z_ps[:], pT[kb][:], x_nat[:, kb, :], start=(kb == 0), stop=(kb == 3)
                )
            z_sb = cpool.tile([128, D], FP32)
            nc.vector.tensor_scalar_mul(z_sb[:], z_ps[:], recip[:])
            nc.sync.dma_start(out_d[:, :], z_sb[:])

    nc.compile()
    return nc


_prog_cache: dict = {}


def _get_program() -> bass.Bass:
    if "nc" not in _prog_cache:
        _prog_cache["nc"] = build_program()
    return _prog_cache["nc"]


def make_in_maps(x, h, c, W1, W2, v):
    x = np.ascontiguousarray(x, np.float32)
    h = np.ascontiguousarray(h, np.float32)
    c = np.ascontiguousarray(c, np.float32)
    W1 = np.ascontiguousarray(W1, np.float32)
    W2 = np.ascontiguousarray(W2, np.float32)
    v = np.ascontiguousarray(v, np.float32)
    in_maps = []
    for s in range(NCORES):
        b, t0 = s // (NCORES // B), TL * (s % (NCORES // B))
        in_maps.append({
            "x": x[b],
            "h": h[b, t0:t0 + TL],
            "c": c[b, t0:t0 + TL],
            "W1": W1, "W2": W2, "v": v,
        })
    return in_maps


def kernel(x, h, c, W1, W2, v):
    nc = _get_program()
    in_maps = make_in_maps(x, h, c, W1, W2, v)
    res = run_bass_kernel_spmd(nc, in_maps, core_ids=list(range(NCORES)))
    outs = [res.results[s]["out"] for s in range(NCORES)]
    z = np.stack([np.concatenate(outs[b * 4:(b + 1) * 4], axis=0) for b in range(B)])
    return z.astype(np.float32)


if __name__ == "__main__":
    rng = np.random.default_rng(0)
    x = rng.standard_normal((B, T, D), dtype=np.float32)
    h = rng.standard_normal((B, T, H), dtype=np.float32)
    c = rng.standard_normal((B, T, H), dtype=np.float32)
    W1 = rng.standard_normal((2 * H, V), dtype=np.float32) / np.sqrt(2 * H)
    W2 = rng.standard_normal((D, V), dtype=np.float32) / np.sqrt(D)
    v = rng.standard_normal((V,), dtype=np.float32)
    z = kernel(x=x, h=h, c=c, W1=W1, W2=W2, v=v)
    print(z.shape, z.dtype)
